# revision 3
# baseline (speedup 1.0000x reference)
"""Distributed multi-head attention on 8 Trainium2 NeuronCores (Bass/Tile).

Problem: x[4,2048,1024] f32; q = x@Wq, kv = x@Wkv, 16 heads x 64;
softmax(q k^T / sqrt(64)) @ v; out @ Wo + bo.

Sharding (no collectives): 8 cores = 4 batches x 2 sequence halves.
Each core computes the full 16-head attention for its (batch, query-half):
K/V are computed over the full sequence (duplicated across the 2 cores of a
batch, ~25% extra FLOPs, zero communication).

Per-core kernel layout (all matmuls contract over the partition dim):
  QT [inner,qtok]  = Wq^T x_q^T      (lhsT=Wq tile, rhs=x_q^T)
  KT [inner,tok]   = Wk^T x^T
  V  [tok,inner]   = x^T^T Wv        (lhsT=x^T tile, rhs=Wv)  + ones column
                     per head (augmented V -> softmax denominator for free)
  scoresT [j,i]    = (K_h Q_h^T)     per head, j=key tile, i=query
  attnT            = exp(scoresT)    (scale folded into Wq on host)
  out_augT [65,i]  = [V_h|1]^T attnT  -> row 64 = softmax denominator
  outT [64,i]      = out_augT[0:64] * (1/denom) broadcast
  y [tok,dim]      = outT^T Wo + bo  (bias via K=1 matmul of ones x bo)
"""

import os
import sys

for _p in ("/opt/trn_rl_repo", "/root/.axon_site/_ro/trn_rl_repo"):
    if os.path.isdir(_p) and _p not in sys.path:
        sys.path.append(_p)

import numpy as np
import ml_dtypes

import concourse.bacc as bacc
import concourse.mybir as mybir
import concourse.tile as tile
from concourse.bass_utils import run_bass_kernel_spmd
from contextlib import ExitStack

P = 128
DIM = 1024
HEADS = 16
DH = 64
NSEQ = 2048
NTOK = 1024  # query tokens per core (sequence half)
KD = DIM // P  # 8 contraction tiles
MI = DIM // P  # 8 inner tiles
SCALE = 1.0 / DH**0.5
N_CORES = 8

CD = mybir.dt.bfloat16
NP_CD = ml_dtypes.bfloat16
F32 = mybir.dt.float32
Exp = mybir.ActivationFunctionType.Exp
MULT = mybir.AluOpType.mult

_CACHE = {}


def build_nc():
    nc = bacc.Bacc(
        "TRN2", target_bir_lowering=False, debug=False, num_devices=N_CORES
    )

    xt_e = nc.dram_tensor("xt", [DIM, NSEQ], CD, kind="ExternalInput")
    xtq_e = nc.dram_tensor("xtq", [DIM, NTOK], CD, kind="ExternalInput")
    wq_e = nc.dram_tensor("wq", [DIM, DIM], CD, kind="ExternalInput")
    wk_e = nc.dram_tensor("wk", [DIM, DIM], CD, kind="ExternalInput")
    wv_e = nc.dram_tensor("wv", [DIM, DIM], CD, kind="ExternalInput")
    wo_e = nc.dram_tensor("wo", [DIM, DIM], CD, kind="ExternalInput")
    bo_e = nc.dram_tensor("bo", [1, DIM], CD, kind="ExternalInput")
    out_e = nc.dram_tensor("out", [NTOK, DIM], F32, kind="ExternalOutput")

    # DRAM views with the contraction dim on partitions
    xt_r = xt_e.ap().rearrange("(k p) n -> p k n", p=P)
    xtq_r = xtq_e.ap().rearrange("(k p) n -> p k n", p=P)
    wq_r = wq_e.ap().rearrange("(k p) n -> p k n", p=P)
    wk_r = wk_e.ap().rearrange("(k p) n -> p k n", p=P)
    wv_r = wv_e.ap().rearrange("(k p) n -> p k n", p=P)
    wo_r = wo_e.ap().rearrange("(k p) n -> p k n", p=P)
    out_r = out_e.ap()

    with tile.TileContext(nc) as tc, ExitStack() as top:
        const = top.enter_context(tc.tile_pool(name="const", bufs=1))
        wo_p = top.enter_context(tc.tile_pool(name="wo_p", bufs=1))
        qt_p = top.enter_context(tc.tile_pool(name="qt_p", bufs=1))
        kt_p = top.enter_context(tc.tile_pool(name="kt_p", bufs=1))
        vg_p = top.enter_context(tc.tile_pool(name="vg_p", bufs=1))
        ot_p = top.enter_context(tc.tile_pool(name="ot_p", bufs=1))
        at_p = top.enter_context(tc.tile_pool(name="at_p", bufs=2))
        dn_p = top.enter_context(tc.tile_pool(name="dn_p", bufs=1))
        rb_p = top.enter_context(tc.tile_pool(name="rb_p", bufs=1))
        y_p = top.enter_context(tc.tile_pool(name="y_p", bufs=2))
        ps_p = top.enter_context(tc.tile_pool(name="ps_p", bufs=2, space="PSUM"))
        po_p = top.enter_context(tc.tile_pool(name="po_p", bufs=2, space="PSUM"))

        ones128 = const.tile([1, P], CD)
        ones64 = const.tile([1, DH], F32)
        bo_sb = const.tile([1, DIM], CD)
        nc.vector.memset(ones128[:], 1.0)
        nc.vector.memset(ones64[:], 1.0)
        nc.sync.dma_start(out=bo_sb[:], in_=bo_e.ap())

        wo_sb = wo_p.tile([P, KD, DIM], CD)
        for k in range(KD):
            nc.sync.dma_start(out=wo_sb[:, k, :], in_=wo_r[:, k, :])

        QT = qt_p.tile([P, MI, NTOK], CD)
        KT = kt_p.tile([P, MI, NSEQ], CD)
        Vg = vg_p.tile([P, NSEQ // P, HEADS * (DH + 1)], CD)
        outT = ot_p.tile([P, MI, NTOK], CD)

        # augmented-V ones columns (col 64 of each head's 65-wide group)
        vg4 = Vg.rearrange("p t (h c) -> p t h c", c=DH + 1)
        nc.vector.memset(vg4[:, :, :, DH : DH + 1], 1.0)

        # ---- phase A: QT = Wq^T @ xq^T ----
        with ExitStack() as es_a:
            a_pool = es_a.enter_context(tc.tile_pool(name="a_pool", bufs=1))
            wq_sb = a_pool.tile([P, KD, DIM], CD)
            xtq_sb = a_pool.tile([P, KD, NTOK], CD)
            for k in range(KD):
                nc.sync.dma_start(out=wq_sb[:, k, :], in_=wq_r[:, k, :])
            for k in range(KD):
                nc.sync.dma_start(out=xtq_sb[:, k, :], in_=xtq_r[:, k, :])

            for m in range(MI):
                ps = ps_p.tile([P, 1024], F32, name="ps", tag="ps")
                for n in range(2):
                    for k in range(KD):
                        nc.tensor.matmul(
                            ps[:, n * 512 : (n + 1) * 512],
                            wq_sb[:, k, m * P : (m + 1) * P],
                            xtq_sb[:, k, n * 512 : (n + 1) * 512],
                            start=(k == 0),
                            stop=(k == KD - 1),
                        )
                nc.scalar.copy(QT[:, m, :], ps[:])

        # ---- phase B: KT (m0 first), V, then KT m1.. interleaved with C ----
        es_b = ExitStack()
        b_pool = es_b.enter_context(tc.tile_pool(name="b_pool", bufs=1))
        wkm_p = es_b.enter_context(tc.tile_pool(name="wkm_p", bufs=2))
        xt_sb = b_pool.tile([P, KD, NSEQ], CD)
        wv_sb = b_pool.tile([P, KD, DIM], CD)
        for k in range(KD):
            nc.sync.dma_start(out=xt_sb[:, k, :], in_=xt_r[:, k, :])
        for k in range(KD):
            nc.sync.dma_start(out=wv_sb[:, k, :], in_=wv_r[:, k, :])

        def emit_kt(m):
            # stream the m-th column block of Wk
            wk_m = wkm_p.tile([P, KD, P], CD, name="wk_m", tag="wk_m")
            for k in range(KD):
                nc.sync.dma_start(
                    out=wk_m[:, k, :], in_=wk_r[:, k, m * P : (m + 1) * P]
                )
            for half in range(2):
                ps = ps_p.tile([P, 1024], F32, name="ps", tag="ps")
                for n in range(2):
                    c0 = half * 1024 + n * 512
                    for k in range(KD):
                        nc.tensor.matmul(
                            ps[:, n * 512 : (n + 1) * 512],
                            wk_m[:, k, :],
                            xt_sb[:, k, c0 : c0 + 512],
                            start=(k == 0),
                            stop=(k == KD - 1),
                        )
                nc.scalar.copy(KT[:, m, half * 1024 : (half + 1) * 1024], ps[:])

        emit_kt(0)

        # V (natural layout, written into augmented positions)
        vg_dst = vg4[:, :, :, 0:DH]  # [p, t, h, 64]
        for t in range(NSEQ // P):
            ps = ps_p.tile([P, 1024], F32, name="ps", tag="ps")
            for n in range(2):
                for k in range(KD):
                    nc.tensor.matmul(
                        ps[:, n * 512 : (n + 1) * 512],
                        xt_sb[:, k, t * P : (t + 1) * P],
                        wv_sb[:, k, n * 512 : (n + 1) * 512],
                        start=(k == 0),
                        stop=(k == KD - 1),
                    )
            ps_h = ps.rearrange("p (h c) -> p h c", c=DH)
            nc.vector.tensor_copy(vg_dst[:, t, :, :], ps_h[:, :, :])

        # ---- phase C: attention per head pair ----
        for pair in range(HEADS // 2):
            if pair > 0:
                emit_kt(pair)
            for sub in range(2):
                h = pair * 2 + sub
                hb = sub * DH
                m = pair
                po = po_p.tile([DH + 1, NTOK], F32, name="po", tag="po")
                for jt in range(NSEQ // P):
                    ps = ps_p.tile([P, 1024], F32, name="ps", tag="ps")
                    for n in range(2):
                        nc.tensor.matmul(
                            ps[:, n * 512 : (n + 1) * 512],
                            KT[hb : hb + DH, m, jt * P : (jt + 1) * P],
                            QT[hb : hb + DH, m, n * 512 : (n + 1) * 512],
                            start=True,
                            stop=True,
                        )
                    at = at_p.tile([P, NTOK], CD, name="at", tag="at")
                    nc.scalar.activation(at[:], ps[:], Exp)
                    for n in range(2):
                        nc.tensor.matmul(
                            po[:, n * 512 : (n + 1) * 512],
                            Vg[:, jt, h * (DH + 1) : (h + 1) * (DH + 1)],
                            at[:, n * 512 : (n + 1) * 512],
                            start=(jt == 0),
                            stop=(jt == NSEQ // P - 1),
                        )
                # normalize: outT_h = po[0:64] / po[64]
                den = dn_p.tile([1, NTOK], F32, name="den", tag="den")
                nc.vector.tensor_copy(den[:], po[DH : DH + 1, :])
                rec = dn_p.tile([1, NTOK], F32, name="rec", tag="rec")
                nc.vector.reciprocal_approx_fast(rec[:], den[:])
                bc = ps_p.tile([P, 1024], F32, name="ps", tag="ps")
                for n in range(2):
                    nc.tensor.matmul(
                        bc[0:DH, n * 512 : (n + 1) * 512],
                        ones64[:, :],
                        rec[:, n * 512 : (n + 1) * 512],
                        start=True,
                        stop=True,
                    )
                rbc = rb_p.tile([DH, NTOK], F32, name="rbc", tag="rbc")
                nc.vector.tensor_copy(rbc[:], bc[0:DH, :])
                nc.vector.tensor_tensor(
                    outT[hb : hb + DH, m, :], po[0:DH, :], rbc[:], MULT
                )
        es_b.close()

        # ---- phase D: y = outT^T @ Wo + bo ----
        for t in range(NTOK // P):
            ps = ps_p.tile([P, 1024], F32, name="ps", tag="ps")
            for n in range(2):
                nc.tensor.matmul(
                    ps[:, n * 512 : (n + 1) * 512],
                    ones128[:, :],
                    bo_sb[:, n * 512 : (n + 1) * 512],
                    start=True,
                    stop=False,
                )
                for k in range(KD):
                    nc.tensor.matmul(
                        ps[:, n * 512 : (n + 1) * 512],
                        outT[:, k, t * P : (t + 1) * P],
                        wo_sb[:, k, n * 512 : (n + 1) * 512],
                        start=False,
                        stop=(k == KD - 1),
                    )
            y = y_p.tile([P, DIM], F32, name="y", tag="y")
            nc.vector.tensor_copy(y[:], ps[:])
            nc.sync.dma_start(out=out_r[t * P : (t + 1) * P, :], in_=y[:])

    nc.compile()
    return nc


def make_in_maps(x, Wq, Wkv, Wo, bo):
    x = np.asarray(x, dtype=np.float32)
    wq_s = (np.asarray(Wq, dtype=np.float32) * SCALE).astype(NP_CD)
    wk = np.ascontiguousarray(np.asarray(Wkv, np.float32)[:, :DIM]).astype(NP_CD)
    wv = np.ascontiguousarray(np.asarray(Wkv, np.float32)[:, DIM:]).astype(NP_CD)
    wo = np.asarray(Wo, dtype=np.float32).astype(NP_CD)
    bo2 = np.asarray(bo, dtype=np.float32).reshape(1, DIM).astype(NP_CD)

    in_maps = []
    for core in range(N_CORES):
        b, s = core // 2, core % 2
        xt = np.ascontiguousarray(x[b].T).astype(NP_CD)
        xtq = np.ascontiguousarray(xt[:, s * NTOK : (s + 1) * NTOK])
        in_maps.append(
            {
                "xt": xt,
                "xtq": xtq,
                "wq": wq_s,
                "wk": wk,
                "wv": wv,
                "wo": wo,
                "bo": bo2,
            }
        )
    return in_maps


def kernel(x, Wq, Wkv, Wo, bo):
    if "nc" not in _CACHE:
        _CACHE["nc"] = build_nc()
    nc = _CACHE["nc"]
    in_maps = make_in_maps(x, Wq, Wkv, Wo, bo)
    res = run_bass_kernel_spmd(nc, in_maps, core_ids=list(range(N_CORES)))
    out = np.empty((4, NSEQ, DIM), dtype=np.float32)
    for core in range(N_CORES):
        b, s = core // 2, core % 2
        out[b, s * NTOK : (s + 1) * NTOK, :] = res.results[core]["out"]
    return out


if __name__ == "__main__":
    # quick self-run with random data
    rng = np.random.default_rng(0)
    x = rng.standard_normal((4, NSEQ, DIM), dtype=np.float32)
    Wq = rng.standard_normal((DIM, DIM), dtype=np.float32) / 32
    Wkv = rng.standard_normal((DIM, 2 * DIM), dtype=np.float32) / 32
    Wo = rng.standard_normal((DIM, DIM), dtype=np.float32) / 32
    bo = rng.standard_normal((DIM,), dtype=np.float32) * 0.01
    out = kernel(x=x, Wq=Wq, Wkv=Wkv, Wo=Wo, bo=bo)
    print("out", out.shape, out.dtype, np.abs(out).mean())
